# revision 11
# baseline (speedup 1.0000x reference)
"""Trainium2 Bass kernel for nn_Attention_48661979463892.

Multi-head attention: B=2, H=8, dk=dv=64, T=S=2048, E=512.
  keys    = Wk @ x[b]          -> per head [64, T]
  values  = Wv @ x[b]          -> per head [64, T]
  queries = Wq @ y[b]          -> per head [64, S]
  scores  = keys^T @ queries + mask            [T, S]
  attn    = softmax(0.125 * scores, axis=T)    (normalize over keys axis)
  out     = values @ attn                      [64, S]
  res     = W @ concat_heads(out) + b          -> [B, S, O]

Sharding: 16 (batch, head) pairs over 8 cores -> core c handles batch c//4,
head-pair c%4 (global head rows 128*(c%4) .. +128).  Each core emits a
partial [S, O] contribution of the final linear (its 128 v-channels); the
host sums 4 partials per batch and adds the bias.

v2 structure (vs the 112us v1 baseline):
  * fp16 matmul operands everywhere (more mantissa than bf16, and it
    enables the bit-trick exp below).
  * t-major input layout: x and y arrive as four [128, 4j, 512] t-slices,
    so the first projection chunk only needs 1/4 of x -> the scores/exp
    pipeline starts ~13us earlier.  All input DMAs are issued up front in
    priority order (weights, x/y slice 0 on sync; the rest on gpsimd).
  * The exp of the scores (the ACT-engine bottleneck: 64 tiles of
    [128,1024] at ~1.1us each) is split between ACT and the Vector engine.
    DVE tiles use a Schraudolph-style bit-trick: y = int16(round(A*s + B))
    reinterpreted as fp16 approximates exp(0.125*s) to ~1.8% rms; one
    tensor_scalar op per tile, and the int16 tile *is* the fp16 AV-matmul
    rhs via a bitcast view.  ~28% of tiles on DVE balances the engines and
    keeps the extra error at ~0.9% fro (gate is 2e-2).
  * scores matmuls are row-tiled (K=64 head pairs at tile rows 0/64) and
    run concurrently on the PE.
  * softmax denominator via a ones-column appended to values^T (M=65 AV
    matmuls); the 1/colsum scales are transposed into per-partition
    columns with K=1 matmuls and fused into the final-linear PSUM drains.
  * fp16 result store (host accumulates partials in f32).
"""

import numpy as np

N_CORES = 8
B, I, T, S, O = 2, 512, 2048, 512, 512
S = 2048
H_PER_CORE = 2
DK = 64
SCALING = DK ** -0.5  # 0.125

# Schraudolph fp16 exp: exp(0.125*s) ~= bitcast_f16(int16(EXP_A*s + EXP_B))
EXP_A = float(1024.0 / np.log(2.0) * SCALING)   # 184.66494
EXP_B = float(15.0 * 1024.0 - 60.0)             # C=60 calibrated offline

N_WARMUP_MM = 16

# which t-tiles' exp goes to the Vector engine (per s-chunk)
DVE_TTS = (
    (8, 12),              # sc0 is PE-heavy (inline projections) and DVE
    (2, 5, 8, 11, 14),    # does the valT drains, so fewer tiles there
    (2, 5, 8, 11, 14),
    (1, 4, 7, 10, 13),    # early in sc3 so DVE is free for the tail
)

_BUILD_CACHE = {}


def _split_multi_waits(nc):
    """walrus in this toolchain accepts only ONE sync wait per instruction.
    Hoist extra waits onto same-engine NoOps inserted just before."""
    import concourse.mybir as mybir

    ctr = 0
    for fn in nc.m.functions:
        for blk in fn.blocks:
            new_insts = []
            for inst in blk.instructions:
                si = inst.sync_info
                if si is not None and len(si.on_wait) > 1:
                    waits = list(si.on_wait)
                    for w in waits[:-1]:
                        ctr += 1
                        nop = mybir.InstNoOp(
                            name=f"waitsplit-{ctr}", ins=[], outs=[]
                        )
                        nop.engine = inst.engine
                        nop.sync_info = mybir.SyncInfo(on_wait=[w], on_update=[])
                        new_insts.append(nop)
                    del si.on_wait[:-1]
                new_insts.append(inst)
            blk.instructions[:] = new_insts


def _build(with_mask):
    import concourse.bass as bass
    import concourse.mybir as mybir
    import concourse.tile as tile
    from concourse.bass import ts, ds

    f32 = mybir.dt.float32
    f16 = mybir.dt.float16
    i16 = mybir.dt.int16
    nc = bass.Bass()
    # x4/y4: [n(4), 128, j(4), 512] t-major slices
    x_p = nc.declare_dram_parameter("x4", [4, 128, 4, 512], f16, isOutput=False)
    y_p = nc.declare_dram_parameter("y4", [4, 128, 4, 512], f16, isOutput=False)
    wk_p = nc.declare_dram_parameter("wkT", [128, 4, 128], f16, isOutput=False)
    wv_p = nc.declare_dram_parameter("wvT", [128, 4, 128], f16, isOutput=False)
    wq_p = nc.declare_dram_parameter("wqT", [128, 4, 128], f16, isOutput=False)
    wc_p = nc.declare_dram_parameter("wcT", [2, 64, O], f16, isOutput=False)
    if with_mask:
        mask_p = nc.declare_dram_parameter("maskT", [16, 128, S], f32, isOutput=False)
    res_p = nc.declare_dram_parameter("res", [S, O], f16, isOutput=True)

    N_SC = S // 512    # s chunks of 512
    N_TT = T // 128    # t tiles of 128

    with tile.TileContext(nc) as tc:
        with (
            nc.allow_low_precision(reason="fp16 matmul operands + bit-trick exp"),
            tc.tile_pool(name="consts", bufs=1) as consts,
            tc.tile_pool(name="exps", bufs=4) as exps_pool,
            tc.tile_pool(name="epi", bufs=2) as epi_pool,
            tc.tile_pool(name="osb", bufs=4) as osb_pool,
            tc.tile_pool(name="resout", bufs=3) as res_pool,
            tc.tile_pool(name="ps_scores", bufs=3, space="PSUM") as ps_scores_pool,
            tc.tile_pool(name="ps_acc", bufs=2, space="PSUM") as ps_acc_pool,
        ):
            # dummy matmuls on scratch data keep the PE busy while the input
            # DMAs land, so the HAM clock-gate is warm when real work starts
            scratch_sb = consts.tile([128, 512], f16)
            nc.vector.memset(scratch_sb, 0.0)
            for w in range(N_WARMUP_MM):
                ps_w = ps_scores_pool.tile([128, 1024], f32, tag="ps_s", name="ps_w")
                nc.tensor.matmul(
                    ps_w[:, 0:512], scratch_sb[:, 0:128], scratch_sb,
                    start=True, stop=True,
                )

            # ---------------- load inputs ----------------
            wk_sb = consts.tile([128, 4, 128], f16)
            wv_sb = consts.tile([128, 4, 128], f16)
            wq_sb = consts.tile([128, 4, 128], f16)
            wc_sb0 = consts.tile([64, O], f16)
            wc_sb1 = consts.tile([64, O], f16)
            x_sb = consts.tile([128, 4, 4, 512], f16)   # [p, n, j, s]
            y_sb = consts.tile([128, 4, 4, 512], f16)
            # each HW-DGE ring sustains only ~170 GB/s when both are busy,
            # so split the critical head transfers across rings: x rides
            # sync, y + the other weights ride gpsimd, earliest-needed first
            nc.sync.dma_start(out=wk_sb, in_=wk_p[:, :, :])
            nc.sync.dma_start(out=x_sb[:, 0, 0:2], in_=x_p[0][:, 0:2])
            nc.sync.dma_start(out=y_sb[:, 0, 0:2], in_=y_p[0][:, 0:2])
            nc.sync.dma_start(out=y_sb[:, 0, 2:4], in_=y_p[0][:, 2:4])
            nc.sync.dma_start(out=x_sb[:, 1], in_=x_p[1])
            nc.sync.dma_start(out=x_sb[:, 2], in_=x_p[2])
            nc.gpsimd.dma_start(out=wq_sb, in_=wq_p[:, :, :])
            nc.gpsimd.dma_start(out=x_sb[:, 0, 2:4], in_=x_p[0][:, 2:4])
            nc.gpsimd.dma_start(out=wv_sb, in_=wv_p[:, :, :])
            nc.gpsimd.dma_start(out=x_sb[:, 3], in_=x_p[3])
            nc.gpsimd.dma_start(out=y_sb[:, 1], in_=y_p[1])
            nc.gpsimd.dma_start(out=y_sb[:, 2], in_=y_p[2])
            nc.gpsimd.dma_start(out=y_sb[:, 3], in_=y_p[3])
            nc.gpsimd.dma_start(out=wc_sb0, in_=wc_p[0])
            nc.gpsimd.dma_start(out=wc_sb1, in_=wc_p[1])

            # ---------------- projections ----------------
            keys_sb = consts.tile([128, T], f16)
            qs_sb = consts.tile([128, S], f16)

            def project(dst, w_sb, src, n):
                """one 512-wide t-slice: 4 K=128 accumulation matmuls."""
                ps = ps_scores_pool.tile(
                    [128, 1024], f32, tag="ps_s", name="pj"
                )[:, 0:512]
                for j in range(4):
                    nc.tensor.matmul(
                        ps, w_sb[:, j, :], src[:, n, j, :],
                        start=(j == 0), stop=(j == 3),
                    )
                nc.vector.tensor_copy(out=dst[:, ts(n, 512)], in_=ps)

            # values^T with ones columns: [t_part=128, tt, 130]
            # cols 0:64 head0, col 64 ones, cols 65:129 head1, col 129 ones.
            valT_sb = consts.tile([128, N_TT, 130], f16)
            nc.vector.memset(valT_sb[:, :, 64:65], 1.0)
            nc.vector.memset(valT_sb[:, :, 129:130], 1.0)

            def valT_proj(tt):
                ps = ps_scores_pool.tile(
                    [128, 1024], f32, tag="ps_s", name="pv"
                )[:, 0:128]
                for j in range(4):
                    nc.tensor.matmul(
                        ps,
                        x_sb[:, tt // 4, j, ts(tt % 4, 128)],
                        wv_sb[:, j, :],
                        start=(j == 0), stop=(j == 3),
                    )
                nc.vector.tensor_copy(out=valT_sb[:, tt, 0:64], in_=ps[:, 0:64])
                nc.vector.tensor_copy(out=valT_sb[:, tt, 65:129], in_=ps[:, 64:128])

            def scores_mm(sc, tt):
                """row-tiled pair of K=64 score matmuls into one PSUM pair."""
                ps_s = ps_scores_pool.tile([128, 1024], f32, tag="ps_s", name="ps_s")
                if with_mask:
                    m_sb = exps_pool.tile([128, 512], f32, tag="mask", name="m_sb")
                    nc.sync.dma_start(out=m_sb, in_=mask_p[tt][:, ts(sc, 512)])
                for h in range(2):
                    nc.tensor.matmul(
                        ps_s[:, ts(h, 512)],
                        keys_sb[64 * h : 64 * h + 64, ts(tt, 128)],
                        qs_sb[64 * h : 64 * h + 64, ts(sc, 512)],
                        start=True,
                        stop=True,
                    )
                    if with_mask:
                        nc.vector.tensor_tensor(
                            ps_s[:, ts(h, 512)],
                            ps_s[:, ts(h, 512)],
                            m_sb,
                            mybir.AluOpType.add,
                        )
                return ps_s

            def exp_tile(sc, tt, ps_s):
                if tt in DVE_TTS[sc]:
                    # bit-trick exp on the Vector engine: the int16
                    # result bitcasts to the fp16 AV operand directly
                    ex_i = exps_pool.tile([128, 1024], i16, tag="ex", name="ex")
                    nc.vector.tensor_scalar(
                        out=ex_i,
                        in0=ps_s,
                        scalar1=EXP_A,
                        scalar2=EXP_B,
                        op0=mybir.AluOpType.mult,
                        op1=mybir.AluOpType.add,
                    )
                    return ex_i.bitcast(f16)
                ex_f = exps_pool.tile([128, 1024], f16, tag="ex", name="ex")
                nc.scalar.activation(
                    out=ex_f,
                    in_=ps_s,
                    func=mybir.ActivationFunctionType.Exp,
                    scale=float(SCALING),
                )
                return ex_f

            # only what the first scores tile needs; the rest of the
            # projections are interleaved into sc0's t-loop
            project(keys_sb, wk_sb, x_sb, 0)
            valT_proj(0)
            valT_proj(1)
            project(qs_sb, wq_sb, y_sb, 0)

            def extra_work(sc, tt):
                if sc == 0:
                    if tt + 2 < N_TT:
                        valT_proj(tt + 2)
                    if tt in (1, 3, 5):        # keys n1/n2/n3 (needed by the
                        project(keys_sb, wk_sb, x_sb, (tt + 1) // 2)  # scores
                    elif tt == 7:              # 2-ahead at tt 2/4/8)
                        project(qs_sb, wq_sb, y_sb, 1)
                elif sc in (1, 2) and tt == 7:  # queries for sc2/sc3
                    project(qs_sb, wq_sb, y_sb, sc + 1)

            # ---------------- attention main loop (software-pipelined) ----
            # The scores pair for tile tt+2 is emitted BEFORE the AV of tile
            # tt, so the exp engines always have a ready tile and run
            # back-to-back; the PE work (scores+AV+projections) hides under
            # them.  The previous chunk's normalize+final-linear is spread
            # over the first iterations.
            def t_loop(sc, prev_osb):
                osc_prev = None
                ps_o = [
                    ps_acc_pool.tile([65, 512], f32, tag="av", name=f"ps_o{h}")
                    for h in range(2)
                ]
                pend = {0: scores_mm(sc, 0), 1: scores_mm(sc, 1)}
                ex_ready = {}
                for tt in range(N_TT):
                    if tt in ex_ready:
                        ex = ex_ready.pop(tt)
                    else:
                        ex = exp_tile(sc, tt, pend.pop(tt))
                    if tt + 2 < N_TT:
                        pend[tt + 2] = scores_mm(sc, tt + 2)
                    # DVE exp tiles are emitted one iteration early (their
                    # scores landed two iterations ago), so the Vector
                    # engine has a full period of slack before the AV
                    # matmuls block on the result
                    if tt + 1 < N_TT and (tt + 1) in DVE_TTS[sc]:
                        ex_ready[tt + 1] = exp_tile(sc, tt + 1, pend.pop(tt + 1))
                    for h in range(2):
                        nc.tensor.matmul(
                            ps_o[h],
                            valT_sb[:, tt, 65 * h : 65 * h + 65],
                            ex[:, ts(h, 512)],
                            start=(tt == 0),
                            stop=(tt == N_TT - 1),
                        )
                    extra_work(sc, tt)
                    if prev_osb is not None:
                        if tt == 1:
                            osc_prev = normalize(prev_osb)
                        elif tt in (2, 4, 6, 8):
                            epilogue_st(sc - 1, prev_osb, osc_prev, (tt - 2) // 2)
                osb = []
                for h in range(2):
                    o_un = osb_pool.tile([65, 512], f16, tag=f"osb{h}", name=f"osb{h}")
                    if sc == N_SC - 1 and h == 1:
                        nc.scalar.copy(o_un, ps_o[h])
                    else:
                        nc.vector.tensor_copy(out=o_un, in_=ps_o[h])
                    osb.append(o_un)
                return osb, osc_prev

            def normalize(osb):
                """1/colsum as per-partition columns: transpose each [1,128]
                colsum slice into a PSUM column via a K=1 matmul, then one
                tiny [128, 8] reciprocal.  Column h*4+st holds head h,
                s-subtile st."""
                cs_ps = ps_scores_pool.tile(
                    [128, 1024], f32, tag="ps_s", name="cs_ps"
                )[:, 0:8]
                one_mm = valT_sb[64:65, 0, 64:65]
                for h in range(2):
                    for st in range(4):
                        nc.tensor.matmul(
                            cs_ps[:, h * 4 + st : h * 4 + st + 1],
                            osb[h][64:65, ts(st, 128)],
                            one_mm,
                            start=True,
                            stop=True,
                        )
                rec_col = epi_pool.tile([128, 8], f32, tag="rec", name="rec_col")
                nc.vector.reciprocal(out=rec_col, in_=cs_ps)
                return rec_col

            def epilogue_st(sc, osb, rec_col, st):
                """one 128-row s-subtile of the final linear, the 1/colsum
                scaling fused into the PSUM->SBUF drain; store.  For the last
                chunk the scores PSUM pool is dead, so fin pairs borrow its
                banks (deeper pipelining) and the idle ACT takes half the
                scales."""
                last = sc == N_SC - 1
                pr = ps_scores_pool.tile(
                    [128, 1024], f32, tag="ps_s", name="ps_rp"
                )
                ps_r0 = pr[:, 0:512]
                ps_r1 = pr[:, 512:1024]
                nc.tensor.matmul(
                    ps_r0, osb[0][0:64, ts(st, 128)], wc_sb0,
                    start=True, stop=True,
                )
                nc.tensor.matmul(
                    ps_r1, osb[1][0:64, ts(st, 128)], wc_sb1,
                    start=True, stop=True,
                )
                a_sb = res_pool.tile([128, O], f16, tag="a_sb", name="a_sb")
                if last:
                    nc.scalar.activation(
                        out=a_sb,
                        in_=ps_r0,
                        func=mybir.ActivationFunctionType.Copy,
                        scale=rec_col[:, st : st + 1],
                    )
                else:
                    nc.vector.tensor_scalar_mul(
                        a_sb, ps_r0, rec_col[:, st : st + 1]
                    )
                r_sb = res_pool.tile([128, O], f16)
                nc.vector.scalar_tensor_tensor(
                    out=r_sb,
                    in0=ps_r1,
                    scalar=rec_col[:, 4 + st : 5 + st],
                    in1=a_sb,
                    op0=mybir.AluOpType.mult,
                    op1=mybir.AluOpType.add,
                )
                nc.sync.dma_start(
                    out=res_p[ds(sc * 512 + st * 128, 128), :], in_=r_sb
                )

            prev_osb = None
            for sc in range(N_SC):
                prev_osb, _ = t_loop(sc, prev_osb)
            rec_last = normalize(prev_osb)
            for st in range(4):
                epilogue_st(N_SC - 1, prev_osb, rec_last, st)

    _split_multi_waits(nc)
    return nc


def _get_nc(with_mask):
    if with_mask not in _BUILD_CACHE:
        _BUILD_CACHE[with_mask] = _build(with_mask)
    return _BUILD_CACHE[with_mask]


def _make_in_maps(x, y, mask, Wk, Wv, Wq, W, with_mask):
    mdt = np.float16
    in_maps = []
    for c in range(N_CORES):
        bb, hp = divmod(c, 4)
        e_sl = slice(128 * hp, 128 * hp + 128)
        im = {
            "x4": np.ascontiguousarray(
                x[bb].reshape(4, 128, 4, 512).transpose(2, 1, 0, 3).astype(mdt)
            ),
            "y4": np.ascontiguousarray(
                y[bb].reshape(4, 128, 4, 512).transpose(2, 1, 0, 3).astype(mdt)
            ),
            "wkT": np.ascontiguousarray(
                Wk[e_sl].T.reshape(4, 128, 128).transpose(1, 0, 2).astype(mdt)
            ),
            "wvT": np.ascontiguousarray(
                Wv[e_sl].T.reshape(4, 128, 128).transpose(1, 0, 2).astype(mdt)
            ),
            "wqT": np.ascontiguousarray(
                Wq[e_sl].T.reshape(4, 128, 128).transpose(1, 0, 2).astype(mdt)
            ),
            "wcT": np.ascontiguousarray(
                np.stack(
                    [
                        W[:, 128 * hp : 128 * hp + 64].T,
                        W[:, 128 * hp + 64 : 128 * hp + 128].T,
                    ]
                ).astype(mdt)
            ),
        }
        if with_mask:
            im["maskT"] = np.ascontiguousarray(mask.reshape(16, 128, S))
        in_maps.append(im)
    return in_maps


def kernel(x, y, mask, Wk, Wv, Wq, W, b):
    from concourse.bass_utils import run_bass_kernel_spmd

    x = np.asarray(x, dtype=np.float32)
    y = np.asarray(y, dtype=np.float32)
    mask = np.asarray(mask, dtype=np.float32)
    Wk = np.asarray(Wk, dtype=np.float32)
    Wv = np.asarray(Wv, dtype=np.float32)
    Wq = np.asarray(Wq, dtype=np.float32)
    W = np.asarray(W, dtype=np.float32)
    b = np.asarray(b, dtype=np.float32)

    with_mask = bool(np.any(mask))
    nc = _get_nc(with_mask)
    in_maps = _make_in_maps(x, y, mask, Wk, Wv, Wq, W, with_mask)

    r = run_bass_kernel_spmd(nc, in_maps, core_ids=list(range(N_CORES)))
    parts = [r.results[c]["res"].astype(np.float32) for c in range(N_CORES)]
    out = np.stack(
        [
            parts[0] + parts[1] + parts[2] + parts[3],
            parts[4] + parts[5] + parts[6] + parts[7],
        ],
        axis=0,
    )
    out += b[None, None, :]
    return out.astype(np.float32)


# revision 12
# speedup vs baseline: 1.0057x; 1.0057x over previous
"""Trainium2 Bass kernel for nn_Attention_48661979463892.

Multi-head attention: B=2, H=8, dk=dv=64, T=S=2048, E=512.
  keys    = Wk @ x[b]          -> per head [64, T]
  values  = Wv @ x[b]          -> per head [64, T]
  queries = Wq @ y[b]          -> per head [64, S]
  scores  = keys^T @ queries + mask            [T, S]
  attn    = softmax(0.125 * scores, axis=T)    (normalize over keys axis)
  out     = values @ attn                      [64, S]
  res     = W @ concat_heads(out) + b          -> [B, S, O]

Sharding: 16 (batch, head) pairs over 8 cores -> core c handles batch c//4,
head-pair c%4 (global head rows 128*(c%4) .. +128).  Each core emits a
partial [S, O] contribution of the final linear (its 128 v-channels); the
host sums 4 partials per batch and adds the bias.

v2 structure (vs the 112us v1 baseline):
  * fp16 matmul operands everywhere (more mantissa than bf16, and it
    enables the bit-trick exp below).
  * t-major input layout: x and y arrive as four [128, 4j, 512] t-slices,
    so the first projection chunk only needs 1/4 of x -> the scores/exp
    pipeline starts ~13us earlier.  All input DMAs are issued up front in
    priority order (weights, x/y slice 0 on sync; the rest on gpsimd).
  * The exp of the scores (the ACT-engine bottleneck: 64 tiles of
    [128,1024] at ~1.1us each) is split between ACT and the Vector engine.
    DVE tiles use a Schraudolph-style bit-trick: y = int16(round(A*s + B))
    reinterpreted as fp16 approximates exp(0.125*s) to ~1.8% rms; one
    tensor_scalar op per tile, and the int16 tile *is* the fp16 AV-matmul
    rhs via a bitcast view.  ~28% of tiles on DVE balances the engines and
    keeps the extra error at ~0.9% fro (gate is 2e-2).
  * scores matmuls are row-tiled (K=64 head pairs at tile rows 0/64) and
    run concurrently on the PE.
  * softmax denominator via a ones-column appended to values^T (M=65 AV
    matmuls); the 1/colsum scales are transposed into per-partition
    columns with K=1 matmuls and fused into the final-linear PSUM drains.
  * fp16 result store (host accumulates partials in f32).
"""

import numpy as np

N_CORES = 8
B, I, T, S, O = 2, 512, 2048, 512, 512
S = 2048
H_PER_CORE = 2
DK = 64
SCALING = DK ** -0.5  # 0.125

# Schraudolph fp16 exp: exp(0.125*s) ~= bitcast_f16(int16(EXP_A*s + EXP_B))
EXP_A = float(1024.0 / np.log(2.0) * SCALING)   # 184.66494
EXP_B = float(15.0 * 1024.0 - 60.0)             # C=60 calibrated offline

N_WARMUP_MM = 16

# which t-tiles' exp goes to the Vector engine (per s-chunk)
DVE_TTS = (
    (8, 12),              # sc0 is PE-heavy (inline projections) and DVE
    (2, 6, 10, 14),       # does the valT drains, so fewer tiles there;
    (2, 6, 10, 14),       # 4/chunk keeps DVE under ~95% so drain jitter
    (1, 5, 9, 13),        # doesn't stall the AV matmuls
)

_BUILD_CACHE = {}


def _split_multi_waits(nc):
    """walrus in this toolchain accepts only ONE sync wait per instruction.
    Hoist extra waits onto same-engine NoOps inserted just before."""
    import concourse.mybir as mybir

    ctr = 0
    for fn in nc.m.functions:
        for blk in fn.blocks:
            new_insts = []
            for inst in blk.instructions:
                si = inst.sync_info
                if si is not None and len(si.on_wait) > 1:
                    waits = list(si.on_wait)
                    for w in waits[:-1]:
                        ctr += 1
                        nop = mybir.InstNoOp(
                            name=f"waitsplit-{ctr}", ins=[], outs=[]
                        )
                        nop.engine = inst.engine
                        nop.sync_info = mybir.SyncInfo(on_wait=[w], on_update=[])
                        new_insts.append(nop)
                    del si.on_wait[:-1]
                new_insts.append(inst)
            blk.instructions[:] = new_insts


def _build(with_mask):
    import concourse.bass as bass
    import concourse.mybir as mybir
    import concourse.tile as tile
    from concourse.bass import ts, ds

    f32 = mybir.dt.float32
    f16 = mybir.dt.float16
    i16 = mybir.dt.int16
    nc = bass.Bass()
    # x4/y4: [n(4), 128, j(4), 512] t-major slices
    x_p = nc.declare_dram_parameter("x4", [4, 128, 4, 512], f16, isOutput=False)
    y_p = nc.declare_dram_parameter("y4", [4, 128, 4, 512], f16, isOutput=False)
    wk_p = nc.declare_dram_parameter("wkT", [128, 4, 128], f16, isOutput=False)
    wv_p = nc.declare_dram_parameter("wvT", [128, 4, 128], f16, isOutput=False)
    wq_p = nc.declare_dram_parameter("wqT", [128, 4, 128], f16, isOutput=False)
    wc_p = nc.declare_dram_parameter("wcT", [2, 64, O], f16, isOutput=False)
    if with_mask:
        mask_p = nc.declare_dram_parameter("maskT", [16, 128, S], f32, isOutput=False)
    res_p = nc.declare_dram_parameter("res", [S, O], f16, isOutput=True)

    N_SC = S // 512    # s chunks of 512
    N_TT = T // 128    # t tiles of 128

    with tile.TileContext(nc) as tc:
        with (
            nc.allow_low_precision(reason="fp16 matmul operands + bit-trick exp"),
            tc.tile_pool(name="consts", bufs=1) as consts,
            tc.tile_pool(name="exps", bufs=4) as exps_pool,
            tc.tile_pool(name="epi", bufs=2) as epi_pool,
            tc.tile_pool(name="osb", bufs=4) as osb_pool,
            tc.tile_pool(name="resout", bufs=3) as res_pool,
            tc.tile_pool(name="ps_scores", bufs=3, space="PSUM") as ps_scores_pool,
            tc.tile_pool(name="ps_acc", bufs=2, space="PSUM") as ps_acc_pool,
        ):
            # dummy matmuls on scratch data keep the PE busy while the input
            # DMAs land, so the HAM clock-gate is warm when real work starts
            scratch_sb = consts.tile([128, 512], f16)
            nc.vector.memset(scratch_sb, 0.0)
            # tiny dummy exp pulls the ~1.3us ACT_TABLE_LOAD off the
            # critical path (it would otherwise run right before the first
            # real exp tile)
            warm_act = consts.tile([128, 1], f16)
            nc.scalar.activation(
                out=warm_act,
                in_=scratch_sb[:, 0:1],
                func=mybir.ActivationFunctionType.Exp,
            )
            for w in range(N_WARMUP_MM):
                ps_w = ps_scores_pool.tile([128, 1024], f32, tag="ps_s", name="ps_w")
                nc.tensor.matmul(
                    ps_w[:, 0:512], scratch_sb[:, 0:128], scratch_sb,
                    start=True, stop=True,
                )

            # ---------------- load inputs ----------------
            wk_sb = consts.tile([128, 4, 128], f16)
            wv_sb = consts.tile([128, 4, 128], f16)
            wq_sb = consts.tile([128, 4, 128], f16)
            wc_sb0 = consts.tile([64, O], f16)
            wc_sb1 = consts.tile([64, O], f16)
            x_sb = consts.tile([128, 4, 4, 512], f16)   # [p, n, j, s]
            y_sb = consts.tile([128, 4, 4, 512], f16)
            # each HW-DGE ring sustains only ~170 GB/s when both are busy,
            # so split the critical head transfers across rings: x rides
            # sync, y + the other weights ride gpsimd, earliest-needed first
            nc.sync.dma_start(out=wk_sb, in_=wk_p[:, :, :])
            nc.sync.dma_start(out=x_sb[:, 0, 0:2], in_=x_p[0][:, 0:2])
            nc.sync.dma_start(out=x_sb[:, 1], in_=x_p[1])
            nc.sync.dma_start(out=x_sb[:, 2], in_=x_p[2])
            nc.scalar.dma_start(out=y_sb[:, 0, 0:2], in_=y_p[0][:, 0:2])
            nc.scalar.dma_start(out=y_sb[:, 0, 2:4], in_=y_p[0][:, 2:4])
            nc.gpsimd.dma_start(out=wq_sb, in_=wq_p[:, :, :])
            nc.gpsimd.dma_start(out=x_sb[:, 0, 2:4], in_=x_p[0][:, 2:4])
            nc.gpsimd.dma_start(out=wv_sb, in_=wv_p[:, :, :])
            nc.gpsimd.dma_start(out=x_sb[:, 3], in_=x_p[3])
            nc.gpsimd.dma_start(out=y_sb[:, 1], in_=y_p[1])
            nc.gpsimd.dma_start(out=y_sb[:, 2], in_=y_p[2])
            nc.gpsimd.dma_start(out=y_sb[:, 3], in_=y_p[3])
            nc.gpsimd.dma_start(out=wc_sb0, in_=wc_p[0])
            nc.gpsimd.dma_start(out=wc_sb1, in_=wc_p[1])

            # ---------------- projections ----------------
            keys_sb = consts.tile([128, T], f16)
            qs_sb = consts.tile([128, S], f16)

            def project(dst, w_sb, src, n):
                """one 512-wide t-slice: 4 K=128 accumulation matmuls."""
                ps = ps_scores_pool.tile(
                    [128, 1024], f32, tag="ps_s", name="pj"
                )[:, 0:512]
                for j in range(4):
                    nc.tensor.matmul(
                        ps, w_sb[:, j, :], src[:, n, j, :],
                        start=(j == 0), stop=(j == 3),
                    )
                nc.vector.tensor_copy(out=dst[:, ts(n, 512)], in_=ps)

            # values^T with ones columns: [t_part=128, tt, 130]
            # cols 0:64 head0, col 64 ones, cols 65:129 head1, col 129 ones.
            valT_sb = consts.tile([128, N_TT, 130], f16)
            nc.vector.memset(valT_sb[:, :, 64:65], 1.0)
            nc.vector.memset(valT_sb[:, :, 129:130], 1.0)

            def valT_proj(tt):
                ps = ps_scores_pool.tile(
                    [128, 1024], f32, tag="ps_s", name="pv"
                )[:, 0:128]
                for j in range(4):
                    nc.tensor.matmul(
                        ps,
                        x_sb[:, tt // 4, j, ts(tt % 4, 128)],
                        wv_sb[:, j, :],
                        start=(j == 0), stop=(j == 3),
                    )
                nc.vector.tensor_copy(out=valT_sb[:, tt, 0:64], in_=ps[:, 0:64])
                nc.vector.tensor_copy(out=valT_sb[:, tt, 65:129], in_=ps[:, 64:128])

            def scores_mm(sc, tt):
                """row-tiled pair of K=64 score matmuls into one PSUM pair."""
                ps_s = ps_scores_pool.tile([128, 1024], f32, tag="ps_s", name="ps_s")
                if with_mask:
                    m_sb = exps_pool.tile([128, 512], f32, tag="mask", name="m_sb")
                    nc.sync.dma_start(out=m_sb, in_=mask_p[tt][:, ts(sc, 512)])
                for h in range(2):
                    nc.tensor.matmul(
                        ps_s[:, ts(h, 512)],
                        keys_sb[64 * h : 64 * h + 64, ts(tt, 128)],
                        qs_sb[64 * h : 64 * h + 64, ts(sc, 512)],
                        start=True,
                        stop=True,
                    )
                    if with_mask:
                        nc.vector.tensor_tensor(
                            ps_s[:, ts(h, 512)],
                            ps_s[:, ts(h, 512)],
                            m_sb,
                            mybir.AluOpType.add,
                        )
                return ps_s

            def exp_tile(sc, tt, ps_s):
                if tt in DVE_TTS[sc]:
                    # bit-trick exp on the Vector engine: the int16
                    # result bitcasts to the fp16 AV operand directly
                    ex_i = exps_pool.tile([128, 1024], i16, tag="ex", name="ex")
                    nc.vector.tensor_scalar(
                        out=ex_i,
                        in0=ps_s,
                        scalar1=EXP_A,
                        scalar2=EXP_B,
                        op0=mybir.AluOpType.mult,
                        op1=mybir.AluOpType.add,
                    )
                    return ex_i.bitcast(f16)
                ex_f = exps_pool.tile([128, 1024], f16, tag="ex", name="ex")
                nc.scalar.activation(
                    out=ex_f,
                    in_=ps_s,
                    func=mybir.ActivationFunctionType.Exp,
                    scale=float(SCALING),
                )
                return ex_f

            # only what the first scores tile needs; the rest of the
            # projections are interleaved into sc0's t-loop
            project(keys_sb, wk_sb, x_sb, 0)
            valT_proj(0)
            valT_proj(1)
            project(qs_sb, wq_sb, y_sb, 0)

            def extra_work(sc, tt):
                if sc == 0:
                    if tt + 2 < N_TT:
                        valT_proj(tt + 2)
                    if tt in (1, 3, 5):        # keys n1/n2/n3 (needed by the
                        project(keys_sb, wk_sb, x_sb, (tt + 1) // 2)  # scores
                    elif tt == 7:              # 2-ahead at tt 2/4/8)
                        project(qs_sb, wq_sb, y_sb, 1)
                elif sc in (1, 2) and tt == 7:  # queries for sc2/sc3
                    project(qs_sb, wq_sb, y_sb, sc + 1)

            # ---------------- attention main loop (software-pipelined) ----
            # The scores pair for tile tt+2 is emitted BEFORE the AV of tile
            # tt, so the exp engines always have a ready tile and run
            # back-to-back; the PE work (scores+AV+projections) hides under
            # them.  The previous chunk's normalize+final-linear is spread
            # over the first iterations.
            def t_loop(sc, prev_osb):
                osc_prev = None
                ps_o = [
                    ps_acc_pool.tile([65, 512], f32, tag="av", name=f"ps_o{h}")
                    for h in range(2)
                ]
                pend = {0: scores_mm(sc, 0), 1: scores_mm(sc, 1)}
                ex_ready = {}
                for tt in range(N_TT):
                    if tt in ex_ready:
                        ex = ex_ready.pop(tt)
                    else:
                        ex = exp_tile(sc, tt, pend.pop(tt))
                    if tt + 2 < N_TT:
                        pend[tt + 2] = scores_mm(sc, tt + 2)
                    # DVE exp tiles are emitted one iteration early (their
                    # scores landed two iterations ago), so the Vector
                    # engine has a full period of slack before the AV
                    # matmuls block on the result
                    if tt + 1 < N_TT and (tt + 1) in DVE_TTS[sc]:
                        ex_ready[tt + 1] = exp_tile(sc, tt + 1, pend.pop(tt + 1))
                    for h in range(2):
                        nc.tensor.matmul(
                            ps_o[h],
                            valT_sb[:, tt, 65 * h : 65 * h + 65],
                            ex[:, ts(h, 512)],
                            start=(tt == 0),
                            stop=(tt == N_TT - 1),
                        )
                    extra_work(sc, tt)
                    if prev_osb is not None:
                        if tt == 1:
                            osc_prev = normalize(prev_osb)
                        elif tt in (2, 4, 6, 8):
                            epilogue_st(sc - 1, prev_osb, osc_prev, (tt - 2) // 2)
                osb = []
                for h in range(2):
                    o_un = osb_pool.tile([65, 512], f16, tag=f"osb{h}", name=f"osb{h}")
                    if sc == N_SC - 1 and h == 1:
                        nc.scalar.copy(o_un, ps_o[h])
                    else:
                        nc.vector.tensor_copy(out=o_un, in_=ps_o[h])
                    osb.append(o_un)
                return osb, osc_prev

            def normalize(osb):
                """1/colsum as per-partition columns: transpose each [1,128]
                colsum slice into a PSUM column via a K=1 matmul, then one
                tiny [128, 8] reciprocal.  Column h*4+st holds head h,
                s-subtile st."""
                cs_ps = ps_scores_pool.tile(
                    [128, 1024], f32, tag="ps_s", name="cs_ps"
                )[:, 0:8]
                one_mm = valT_sb[64:65, 0, 64:65]
                for h in range(2):
                    for st in range(4):
                        nc.tensor.matmul(
                            cs_ps[:, h * 4 + st : h * 4 + st + 1],
                            osb[h][64:65, ts(st, 128)],
                            one_mm,
                            start=True,
                            stop=True,
                        )
                rec_col = epi_pool.tile([128, 8], f32, tag="rec", name="rec_col")
                nc.vector.reciprocal(out=rec_col, in_=cs_ps)
                return rec_col

            def epilogue_st(sc, osb, rec_col, st):
                """one 128-row s-subtile of the final linear, the 1/colsum
                scaling fused into the PSUM->SBUF drain; store.  For the last
                chunk the scores PSUM pool is dead, so fin pairs borrow its
                banks (deeper pipelining) and the idle ACT takes half the
                scales."""
                last = sc == N_SC - 1
                pr = ps_scores_pool.tile(
                    [128, 1024], f32, tag="ps_s", name="ps_rp"
                )
                ps_r0 = pr[:, 0:512]
                ps_r1 = pr[:, 512:1024]
                nc.tensor.matmul(
                    ps_r0, osb[0][0:64, ts(st, 128)], wc_sb0,
                    start=True, stop=True,
                )
                nc.tensor.matmul(
                    ps_r1, osb[1][0:64, ts(st, 128)], wc_sb1,
                    start=True, stop=True,
                )
                a_sb = res_pool.tile([128, O], f16, tag="a_sb", name="a_sb")
                if last:
                    nc.scalar.activation(
                        out=a_sb,
                        in_=ps_r0,
                        func=mybir.ActivationFunctionType.Copy,
                        scale=rec_col[:, st : st + 1],
                    )
                else:
                    nc.vector.tensor_scalar_mul(
                        a_sb, ps_r0, rec_col[:, st : st + 1]
                    )
                r_sb = res_pool.tile([128, O], f16)
                nc.vector.scalar_tensor_tensor(
                    out=r_sb,
                    in0=ps_r1,
                    scalar=rec_col[:, 4 + st : 5 + st],
                    in1=a_sb,
                    op0=mybir.AluOpType.mult,
                    op1=mybir.AluOpType.add,
                )
                nc.sync.dma_start(
                    out=res_p[ds(sc * 512 + st * 128, 128), :], in_=r_sb
                )

            prev_osb = None
            for sc in range(N_SC):
                prev_osb, _ = t_loop(sc, prev_osb)
            rec_last = normalize(prev_osb)
            for st in range(4):
                epilogue_st(N_SC - 1, prev_osb, rec_last, st)

    _split_multi_waits(nc)
    return nc


def _get_nc(with_mask):
    if with_mask not in _BUILD_CACHE:
        _BUILD_CACHE[with_mask] = _build(with_mask)
    return _BUILD_CACHE[with_mask]


def _make_in_maps(x, y, mask, Wk, Wv, Wq, W, with_mask):
    mdt = np.float16
    in_maps = []
    for c in range(N_CORES):
        bb, hp = divmod(c, 4)
        e_sl = slice(128 * hp, 128 * hp + 128)
        im = {
            "x4": np.ascontiguousarray(
                x[bb].reshape(4, 128, 4, 512).transpose(2, 1, 0, 3).astype(mdt)
            ),
            "y4": np.ascontiguousarray(
                y[bb].reshape(4, 128, 4, 512).transpose(2, 1, 0, 3).astype(mdt)
            ),
            "wkT": np.ascontiguousarray(
                Wk[e_sl].T.reshape(4, 128, 128).transpose(1, 0, 2).astype(mdt)
            ),
            "wvT": np.ascontiguousarray(
                Wv[e_sl].T.reshape(4, 128, 128).transpose(1, 0, 2).astype(mdt)
            ),
            "wqT": np.ascontiguousarray(
                Wq[e_sl].T.reshape(4, 128, 128).transpose(1, 0, 2).astype(mdt)
            ),
            "wcT": np.ascontiguousarray(
                np.stack(
                    [
                        W[:, 128 * hp : 128 * hp + 64].T,
                        W[:, 128 * hp + 64 : 128 * hp + 128].T,
                    ]
                ).astype(mdt)
            ),
        }
        if with_mask:
            im["maskT"] = np.ascontiguousarray(mask.reshape(16, 128, S))
        in_maps.append(im)
    return in_maps


def kernel(x, y, mask, Wk, Wv, Wq, W, b):
    from concourse.bass_utils import run_bass_kernel_spmd

    x = np.asarray(x, dtype=np.float32)
    y = np.asarray(y, dtype=np.float32)
    mask = np.asarray(mask, dtype=np.float32)
    Wk = np.asarray(Wk, dtype=np.float32)
    Wv = np.asarray(Wv, dtype=np.float32)
    Wq = np.asarray(Wq, dtype=np.float32)
    W = np.asarray(W, dtype=np.float32)
    b = np.asarray(b, dtype=np.float32)

    with_mask = bool(np.any(mask))
    nc = _get_nc(with_mask)
    in_maps = _make_in_maps(x, y, mask, Wk, Wv, Wq, W, with_mask)

    r = run_bass_kernel_spmd(nc, in_maps, core_ids=list(range(N_CORES)))
    parts = [r.results[c]["res"].astype(np.float32) for c in range(N_CORES)]
    out = np.stack(
        [
            parts[0] + parts[1] + parts[2] + parts[3],
            parts[4] + parts[5] + parts[6] + parts[7],
        ],
        axis=0,
    )
    out += b[None, None, :]
    return out.astype(np.float32)


# revision 14
# speedup vs baseline: 1.0084x; 1.0027x over previous
"""Trainium2 Bass kernel for nn_Attention_48661979463892.

Multi-head attention: B=2, H=8, dk=dv=64, T=S=2048, E=512.
  keys    = Wk @ x[b]          -> per head [64, T]
  values  = Wv @ x[b]          -> per head [64, T]
  queries = Wq @ y[b]          -> per head [64, S]
  scores  = keys^T @ queries + mask            [T, S]
  attn    = softmax(0.125 * scores, axis=T)    (normalize over keys axis)
  out     = values @ attn                      [64, S]
  res     = W @ concat_heads(out) + b          -> [B, S, O]

Sharding: 16 (batch, head) pairs over 8 cores -> core c handles batch c//4,
head-pair c%4 (global head rows 128*(c%4) .. +128).  Each core emits a
partial [S, O] contribution of the final linear (its 128 v-channels); the
host sums 4 partials per batch and adds the bias.

v2 structure (vs the 112us v1 baseline):
  * fp16 matmul operands everywhere (more mantissa than bf16, and it
    enables the bit-trick exp below).
  * t-major input layout: x and y arrive as four [128, 4j, 512] t-slices,
    so the first projection chunk only needs 1/4 of x -> the scores/exp
    pipeline starts ~13us earlier.  All input DMAs are issued up front in
    priority order (weights, x/y slice 0 on sync; the rest on gpsimd).
  * The exp of the scores (the ACT-engine bottleneck: 64 tiles of
    [128,1024] at ~1.1us each) is split between ACT and the Vector engine.
    DVE tiles use a Schraudolph-style bit-trick: y = int16(round(A*s + B))
    reinterpreted as fp16 approximates exp(0.125*s) to ~1.8% rms; one
    tensor_scalar op per tile, and the int16 tile *is* the fp16 AV-matmul
    rhs via a bitcast view.  ~28% of tiles on DVE balances the engines and
    keeps the extra error at ~0.9% fro (gate is 2e-2).
  * scores matmuls are row-tiled (K=64 head pairs at tile rows 0/64) and
    run concurrently on the PE.
  * softmax denominator via a ones-column appended to values^T (M=65 AV
    matmuls); the 1/colsum scales are transposed into per-partition
    columns with K=1 matmuls and fused into the final-linear PSUM drains.
  * fp16 result store (host accumulates partials in f32).
"""

import numpy as np

N_CORES = 8
B, I, T, S, O = 2, 512, 2048, 512, 512
S = 2048
H_PER_CORE = 2
DK = 64
SCALING = DK ** -0.5  # 0.125

# Schraudolph fp16 exp: exp(0.125*s) ~= bitcast_f16(int16(EXP_A*s + EXP_B))
EXP_A = float(1024.0 / np.log(2.0) * SCALING)   # 184.66494
EXP_B = float(15.0 * 1024.0 - 60.0)             # C=60 calibrated offline

N_WARMUP_MM = 16

# which t-tiles' exp goes to the Vector engine (per s-chunk)
DVE_TTS = (
    (8, 12),              # sc0 is PE-heavy (inline projections) and DVE
    (3, 7, 10, 12, 14),   # does the valT drains, so fewer tiles there;
    (3, 7, 10, 12, 14),   # exp tiles avoid the epilogue-cast iterations
    (1, 5, 9, 12, 14),    # (2,4,6,8) so per-iteration DVE load stays low
)

_BUILD_CACHE = {}


def _split_multi_waits(nc):
    """walrus in this toolchain accepts only ONE sync wait per instruction.
    Hoist extra waits onto same-engine NoOps inserted just before."""
    import concourse.mybir as mybir

    ctr = 0
    for fn in nc.m.functions:
        for blk in fn.blocks:
            new_insts = []
            for inst in blk.instructions:
                si = inst.sync_info
                if si is not None and len(si.on_wait) > 1:
                    waits = list(si.on_wait)
                    for w in waits[:-1]:
                        ctr += 1
                        nop = mybir.InstNoOp(
                            name=f"waitsplit-{ctr}", ins=[], outs=[]
                        )
                        nop.engine = inst.engine
                        nop.sync_info = mybir.SyncInfo(on_wait=[w], on_update=[])
                        new_insts.append(nop)
                    del si.on_wait[:-1]
                new_insts.append(inst)
            blk.instructions[:] = new_insts


def _build(with_mask):
    import concourse.bass as bass
    import concourse.mybir as mybir
    import concourse.tile as tile
    from concourse.bass import ts, ds

    f32 = mybir.dt.float32
    f16 = mybir.dt.float16
    i16 = mybir.dt.int16
    nc = bass.Bass()
    # x4/y4: [n(4), 128, j(4), 512] t-major slices
    x_p = nc.declare_dram_parameter("x4", [4, 128, 4, 512], f16, isOutput=False)
    y_p = nc.declare_dram_parameter("y4", [4, 128, 4, 512], f16, isOutput=False)
    wk_p = nc.declare_dram_parameter("wkT", [128, 4, 128], f16, isOutput=False)
    wv_p = nc.declare_dram_parameter("wvT", [128, 4, 128], f16, isOutput=False)
    wq_p = nc.declare_dram_parameter("wqT", [128, 4, 128], f16, isOutput=False)
    wc_p = nc.declare_dram_parameter("wcT", [2, 64, O], f16, isOutput=False)
    if with_mask:
        mask_p = nc.declare_dram_parameter("maskT", [16, 128, S], f32, isOutput=False)
    res_p = nc.declare_dram_parameter("res", [S, 2, O], f16, isOutput=True)
    cs_p = nc.declare_dram_parameter("cs", [2, 4, 512], f16, isOutput=True)

    N_SC = S // 512    # s chunks of 512
    N_TT = T // 128    # t tiles of 128

    with tile.TileContext(nc) as tc:
        with (
            nc.allow_low_precision(reason="fp16 matmul operands + bit-trick exp"),
            tc.tile_pool(name="consts", bufs=1) as consts,
            tc.tile_pool(name="exps", bufs=4) as exps_pool,
            tc.tile_pool(name="osb", bufs=4) as osb_pool,
            tc.tile_pool(name="resout", bufs=3) as res_pool,
            tc.tile_pool(name="ps_scores", bufs=3, space="PSUM") as ps_scores_pool,
            tc.tile_pool(name="ps_acc", bufs=2, space="PSUM") as ps_acc_pool,
        ):
            # dummy matmuls on scratch data keep the PE busy while the input
            # DMAs land, so the HAM clock-gate is warm when real work starts
            scratch_sb = consts.tile([128, 512], f16)
            nc.vector.memset(scratch_sb, 0.0)
            # tiny dummy exp pulls the ~1.3us ACT_TABLE_LOAD off the
            # critical path (it would otherwise run right before the first
            # real exp tile)
            warm_act = consts.tile([128, 1], f16)
            nc.scalar.activation(
                out=warm_act,
                in_=scratch_sb[:, 0:1],
                func=mybir.ActivationFunctionType.Exp,
            )
            for w in range(N_WARMUP_MM):
                ps_w = ps_scores_pool.tile([128, 1024], f32, tag="ps_s", name="ps_w")
                nc.tensor.matmul(
                    ps_w[:, 0:512], scratch_sb[:, 0:128], scratch_sb,
                    start=True, stop=True,
                )

            # ---------------- load inputs ----------------
            wk_sb = consts.tile([128, 4, 128], f16)
            wv_sb = consts.tile([128, 4, 128], f16)
            wq_sb = consts.tile([128, 4, 128], f16)
            wc_sb0 = consts.tile([64, O], f16)
            wc_sb1 = consts.tile([64, O], f16)
            x_sb = consts.tile([128, 4, 4, 512], f16)   # [p, n, j, s]
            y_sb = consts.tile([128, 4, 4, 512], f16)
            # each HW-DGE ring sustains only ~170 GB/s when both are busy,
            # so split the critical head transfers across rings: x rides
            # sync, y + the other weights ride gpsimd, earliest-needed first
            nc.sync.dma_start(out=wk_sb, in_=wk_p[:, :, :])
            nc.sync.dma_start(out=x_sb[:, 0, 0:2], in_=x_p[0][:, 0:2])
            nc.sync.dma_start(out=x_sb[:, 1], in_=x_p[1])
            nc.sync.dma_start(out=x_sb[:, 2], in_=x_p[2])
            nc.scalar.dma_start(out=y_sb[:, 0, 0:2], in_=y_p[0][:, 0:2])
            nc.scalar.dma_start(out=y_sb[:, 0, 2:4], in_=y_p[0][:, 2:4])
            nc.gpsimd.dma_start(out=wq_sb, in_=wq_p[:, :, :])
            nc.gpsimd.dma_start(out=x_sb[:, 0, 2:4], in_=x_p[0][:, 2:4])
            nc.gpsimd.dma_start(out=wv_sb, in_=wv_p[:, :, :])
            nc.gpsimd.dma_start(out=x_sb[:, 3], in_=x_p[3])
            nc.gpsimd.dma_start(out=y_sb[:, 1], in_=y_p[1])
            nc.gpsimd.dma_start(out=y_sb[:, 2], in_=y_p[2])
            nc.gpsimd.dma_start(out=y_sb[:, 3], in_=y_p[3])
            nc.gpsimd.dma_start(out=wc_sb0, in_=wc_p[0])
            nc.gpsimd.dma_start(out=wc_sb1, in_=wc_p[1])

            # ---------------- projections ----------------
            keys_sb = consts.tile([128, T], f16)
            qs_sb = consts.tile([128, S], f16)

            def project(dst, w_sb, src, n):
                """one 512-wide t-slice: 4 K=128 accumulation matmuls."""
                ps = ps_scores_pool.tile(
                    [128, 1024], f32, tag="ps_s", name="pj"
                )[:, 0:512]
                for j in range(4):
                    nc.tensor.matmul(
                        ps, w_sb[:, j, :], src[:, n, j, :],
                        start=(j == 0), stop=(j == 3),
                    )
                nc.vector.tensor_copy(out=dst[:, ts(n, 512)], in_=ps)

            # values^T with ones columns: [t_part=128, tt, 130]
            # cols 0:64 head0, col 64 ones, cols 65:129 head1, col 129 ones.
            valT_sb = consts.tile([128, N_TT, 130], f16)
            nc.vector.memset(valT_sb[:, :, 64:65], 1.0)
            nc.vector.memset(valT_sb[:, :, 129:130], 1.0)

            def valT_proj(tt):
                ps = ps_scores_pool.tile(
                    [128, 1024], f32, tag="ps_s", name="pv"
                )[:, 0:128]
                for j in range(4):
                    nc.tensor.matmul(
                        ps,
                        x_sb[:, tt // 4, j, ts(tt % 4, 128)],
                        wv_sb[:, j, :],
                        start=(j == 0), stop=(j == 3),
                    )
                nc.vector.tensor_copy(out=valT_sb[:, tt, 0:64], in_=ps[:, 0:64])
                nc.vector.tensor_copy(out=valT_sb[:, tt, 65:129], in_=ps[:, 64:128])

            def scores_mm(sc, tt):
                """row-tiled pair of K=64 score matmuls into one PSUM pair."""
                ps_s = ps_scores_pool.tile([128, 1024], f32, tag="ps_s", name="ps_s")
                if with_mask:
                    m_sb = exps_pool.tile([128, 512], f32, tag="mask", name="m_sb")
                    nc.sync.dma_start(out=m_sb, in_=mask_p[tt][:, ts(sc, 512)])
                for h in range(2):
                    nc.tensor.matmul(
                        ps_s[:, ts(h, 512)],
                        keys_sb[64 * h : 64 * h + 64, ts(tt, 128)],
                        qs_sb[64 * h : 64 * h + 64, ts(sc, 512)],
                        start=True,
                        stop=True,
                    )
                    if with_mask:
                        nc.vector.tensor_tensor(
                            ps_s[:, ts(h, 512)],
                            ps_s[:, ts(h, 512)],
                            m_sb,
                            mybir.AluOpType.add,
                        )
                return ps_s

            def exp_tile(sc, tt, ps_s):
                if tt in DVE_TTS[sc]:
                    # bit-trick exp on the Vector engine: the int16
                    # result bitcasts to the fp16 AV operand directly
                    ex_i = exps_pool.tile([128, 1024], i16, tag="ex", name="ex")
                    nc.vector.tensor_scalar(
                        out=ex_i,
                        in0=ps_s,
                        scalar1=EXP_A,
                        scalar2=EXP_B,
                        op0=mybir.AluOpType.mult,
                        op1=mybir.AluOpType.add,
                    )
                    return ex_i.bitcast(f16)
                ex_f = exps_pool.tile([128, 1024], f16, tag="ex", name="ex")
                nc.scalar.activation(
                    out=ex_f,
                    in_=ps_s,
                    func=mybir.ActivationFunctionType.Exp,
                    scale=float(SCALING),
                )
                return ex_f

            # only what the first scores tile needs; the rest of the
            # projections are interleaved into sc0's t-loop
            project(keys_sb, wk_sb, x_sb, 0)
            valT_proj(0)
            valT_proj(1)
            project(qs_sb, wq_sb, y_sb, 0)

            def extra_work(sc, tt):
                if sc == 0:
                    if tt + 2 < N_TT:
                        valT_proj(tt + 2)
                    if tt in (1, 3, 5):        # keys n1/n2/n3 (needed by the
                        project(keys_sb, wk_sb, x_sb, (tt + 1) // 2)  # scores
                    elif tt == 7:              # 2-ahead at tt 2/4/8)
                        project(qs_sb, wq_sb, y_sb, 1)
                elif sc in (1, 2) and tt == 7:  # queries for sc2/sc3
                    project(qs_sb, wq_sb, y_sb, sc + 1)

            # ---------------- attention main loop (software-pipelined) ----
            # The scores pair for tile tt+2 is emitted BEFORE the AV of tile
            # tt, so the exp engines always have a ready tile and run
            # back-to-back; the PE work (scores+AV+projections) hides under
            # them.  The previous chunk's normalize+final-linear is spread
            # over the first iterations.
            def t_loop(sc, prev_osb):
                ps_o = [
                    ps_acc_pool.tile([65, 512], f32, tag="av", name=f"ps_o{h}")
                    for h in range(2)
                ]
                pend = {0: scores_mm(sc, 0), 1: scores_mm(sc, 1)}
                ex_ready = {}
                for tt in range(N_TT):
                    if tt in ex_ready:
                        ex = ex_ready.pop(tt)
                    else:
                        ex = exp_tile(sc, tt, pend.pop(tt))
                    if tt + 2 < N_TT:
                        pend[tt + 2] = scores_mm(sc, tt + 2)
                    # DVE exp tiles are emitted one iteration early (their
                    # scores landed two iterations ago), so the Vector
                    # engine has a full period of slack before the AV
                    # matmuls block on the result
                    if tt + 1 < N_TT and (tt + 1) in DVE_TTS[sc]:
                        ex_ready[tt + 1] = exp_tile(sc, tt + 1, pend.pop(tt + 1))
                    for h in range(2):
                        nc.tensor.matmul(
                            ps_o[h],
                            valT_sb[:, tt, 65 * h : 65 * h + 65],
                            ex[:, ts(h, 512)],
                            start=(tt == 0),
                            stop=(tt == N_TT - 1),
                        )
                    extra_work(sc, tt)
                    if prev_osb is not None and tt in (2, 4, 6, 8):
                        epilogue_st(sc - 1, prev_osb, (tt - 2) // 2)
                osb = []
                for h in range(2):
                    o_un = osb_pool.tile([65, 512], f16, tag=f"osb{h}", name=f"osb{h}")
                    if sc == N_SC - 1 and h == 1:
                        nc.scalar.copy(o_un, ps_o[h])
                    else:
                        nc.vector.tensor_copy(out=o_un, in_=ps_o[h])
                    osb.append(o_un)
                store_cs(sc, osb)
                return osb, None

            def epilogue_st(sc, osb, st):
                """one 128-row s-subtile of the per-head final linear,
                stored unnormalized as [rows, 2, O]; the softmax division by
                the colsums happens on the host.  Both heads drain as a
                single [128, 1024] cast; the last chunk alternates the casts
                between ACT and DVE and borrows the dead scores banks."""
                last = sc == N_SC - 1
                pr = ps_scores_pool.tile(
                    [128, 1024], f32, tag="ps_s", name="ps_rp"
                )
                nc.tensor.matmul(
                    pr[:, 0:512], osb[0][0:64, ts(st, 128)], wc_sb0,
                    start=True, stop=True,
                )
                nc.tensor.matmul(
                    pr[:, 512:1024], osb[1][0:64, ts(st, 128)], wc_sb1,
                    start=True, stop=True,
                )
                r_sb = res_pool.tile([128, 2, 512], f16, tag="r_sb", name="r_sb")
                if last and st % 2 == 1:
                    nc.scalar.copy(r_sb, pr)
                else:
                    nc.vector.tensor_copy(out=r_sb, in_=pr)
                nc.sync.dma_start(
                    out=res_p[ds(sc * 512 + st * 128, 128), :, :], in_=r_sb
                )

            def store_cs(sc, osb):
                for h in range(2):
                    nc.gpsimd.dma_start(out=cs_p[h, sc], in_=osb[h][64:65, :])

            prev_osb = None
            for sc in range(N_SC):
                prev_osb, _ = t_loop(sc, prev_osb)
            for st in range(4):
                epilogue_st(N_SC - 1, prev_osb, st)

    _split_multi_waits(nc)
    return nc


def _get_nc(with_mask):
    if with_mask not in _BUILD_CACHE:
        _BUILD_CACHE[with_mask] = _build(with_mask)
    return _BUILD_CACHE[with_mask]


def _make_in_maps(x, y, mask, Wk, Wv, Wq, W, with_mask):
    mdt = np.float16
    in_maps = []
    for c in range(N_CORES):
        bb, hp = divmod(c, 4)
        e_sl = slice(128 * hp, 128 * hp + 128)
        im = {
            "x4": np.ascontiguousarray(
                x[bb].reshape(4, 128, 4, 512).transpose(2, 1, 0, 3).astype(mdt)
            ),
            "y4": np.ascontiguousarray(
                y[bb].reshape(4, 128, 4, 512).transpose(2, 1, 0, 3).astype(mdt)
            ),
            "wkT": np.ascontiguousarray(
                Wk[e_sl].T.reshape(4, 128, 128).transpose(1, 0, 2).astype(mdt)
            ),
            "wvT": np.ascontiguousarray(
                Wv[e_sl].T.reshape(4, 128, 128).transpose(1, 0, 2).astype(mdt)
            ),
            "wqT": np.ascontiguousarray(
                Wq[e_sl].T.reshape(4, 128, 128).transpose(1, 0, 2).astype(mdt)
            ),
            "wcT": np.ascontiguousarray(
                np.stack(
                    [
                        W[:, 128 * hp : 128 * hp + 64].T,
                        W[:, 128 * hp + 64 : 128 * hp + 128].T,
                    ]
                ).astype(mdt)
            ),
        }
        if with_mask:
            im["maskT"] = np.ascontiguousarray(mask.reshape(16, 128, S))
        in_maps.append(im)
    return in_maps


def kernel(x, y, mask, Wk, Wv, Wq, W, b):
    from concourse.bass_utils import run_bass_kernel_spmd

    x = np.asarray(x, dtype=np.float32)
    y = np.asarray(y, dtype=np.float32)
    mask = np.asarray(mask, dtype=np.float32)
    Wk = np.asarray(Wk, dtype=np.float32)
    Wv = np.asarray(Wv, dtype=np.float32)
    Wq = np.asarray(Wq, dtype=np.float32)
    W = np.asarray(W, dtype=np.float32)
    b = np.asarray(b, dtype=np.float32)

    with_mask = bool(np.any(mask))
    nc = _get_nc(with_mask)
    in_maps = _make_in_maps(x, y, mask, Wk, Wv, Wq, W, with_mask)

    r = run_bass_kernel_spmd(nc, in_maps, core_ids=list(range(N_CORES)))
    out = np.zeros((2, S, O), dtype=np.float32)
    for c in range(N_CORES):
        res = r.results[c]["res"].astype(np.float32)      # [S, 2, O]
        cs = r.results[c]["cs"].astype(np.float32)        # [2, 4, 512]
        inv = 1.0 / cs.reshape(2, S)                      # [2, S]
        out[c // 4] += np.einsum("sho,hs->so", res, inv)
    out += b[None, None, :]
    return out.astype(np.float32)
